# revision 22
# baseline (speedup 1.0000x reference)
"""Multi-head causal self-attention (B=4, T=2048, C=1024, H=16) on 8 TRN2 cores.

Sharding: core c handles batch b = c//2 and head-group hg = c%2 (8 heads):
data parallel over B, tensor parallel over H. Each core computes qk^T for its
heads (xT @ Wqk column-slice, transposed per-head-pair layout), V in natural
layout, causal attention for its 8 heads, and a partial output projection
(row-split W_proj) -> y_partial [T, C]. Host transposes x per core and sums
y[b] = y_partial[2b] + y_partial[2b+1] + b_proj.

Single fused pipeline: the QKV projection for token chunk r+1 is interleaved
as filler work into attention round r (one round per 512-token q chunk), so
the tensor engine never drains between "phases" and stays at max p-state.
Scores for both heads of a pair go into one [128,1024] PSUM tile and share
one exp activation; attn@V emission is software-pipelined two units behind
the score matmuls so it never waits on the scalar engine. Z rows are DMA-
spread straight out of PSUM; normalization uses a 128-lane reciprocal and a
K=2 broadcast matmul, deferred one round and overlapped with the next round.
"""

from collections import deque
from contextlib import ExitStack

import ml_dtypes
import numpy as np

import concourse.bass as bass
import concourse.bacc as bacc
import concourse.mybir as mybir
import concourse.tile as tile
from concourse.bass_utils import run_bass_kernel_spmd
from concourse.masks import make_upper_triangular

B, T, C, H, HS = 4, 2048, 1024, 16, 64
P = 128
NQC = T // 512          # q-chunks of 512
NKB = T // P            # key blocks of 128
SCALE = HS ** -0.5

F32 = mybir.dt.float32
F32R = mybir.dt.float32r
BF16 = mybir.dt.bfloat16
Exp = mybir.ActivationFunctionType.Exp


def build_kernel():
    nc = bacc.Bacc("TRN2", target_bir_lowering=False)

    xt_d = nc.dram_tensor("xt", (C, T), BF16, kind="ExternalInput")
    wqk_d = nc.dram_tensor("wqk", (C, 8 * P), BF16, kind="ExternalInput")
    bqk_d = nc.dram_tensor("bqk", (8 * P,), F32, kind="ExternalInput")
    wv_d = nc.dram_tensor("wv", (C, 512), BF16, kind="ExternalInput")
    bv_d = nc.dram_tensor("bv", (1, 512), F32R, kind="ExternalInput")
    wproj_d = nc.dram_tensor("wproj", (8 * HS, C), F32R, kind="ExternalInput")
    y_d = nc.dram_tensor("y", (T, C), F32, kind="ExternalOutput")

    with tile.TileContext(nc) as tc, ExitStack() as big:
        const = big.enter_context(tc.tile_pool(name="const", bufs=1))
        persist = big.enter_context(tc.tile_pool(name="persist", bufs=1))
        xtp = big.enter_context(tc.tile_pool(name="xtp", bufs=2))
        atp = big.enter_context(tc.tile_pool(name="atp", bufs=5))
        zsp_p = big.enter_context(tc.tile_pool(name="zsp_p", bufs=2))
        rzap = big.enter_context(tc.tile_pool(name="rzap", bufs=2))
        ysp = big.enter_context(tc.tile_pool(name="ysp", bufs=3))
        ps_s = big.enter_context(tc.tile_pool(name="ps_s", bufs=2, space="PSUM"))
        ps_o = big.enter_context(tc.tile_pool(name="ps_o", bufs=1, space="PSUM"))
        ps_x = big.enter_context(tc.tile_pool(name="ps_x", bufs=2, space="PSUM"))

        # ---- tiles ----
        mask = const.tile([P, P], BF16, tag="mask")
        ones_f = const.tile([P, P], F32, tag="ones_f")
        ones_t = const.tile([1, P], F32R, tag="ones")
        selA = const.tile([P, P], F32R, tag="selA")
        selB = const.tile([P, P], F32R, tag="selB")
        bvr = const.tile([1, 512], F32R, tag="bvr")
        # qk_all: 12 blocks of [128, T] bf16; per pair p:
        #   block 3p   = qpadA: rows 0:64 q of head 2p, rows 64:128 zero
        #   block 3p+1 = qpadB: rows 0:64 zero, rows 64:128 q of head 2p+1
        #   block 3p+2 = k pair: rows 0:64 k(2p), 64:128 k(2p+1)
        qk_all = persist.tile([P, 12 * T], BF16, tag="qk")
        # v_all: per (pair, kb): [vA(64) | onesA(1) | vB(64) | onesB(1)] = 130
        v_all = persist.tile([P, 4 * NKB * 130], BF16, tag="v")
        # aoT: pair-stacked [128 = ch(head 2p) | ch(head 2p+1), 4 * T]
        aoT = persist.tile([P, 4 * T], F32R, tag="aoT")
        wqk_sb = persist.tile([P, 8 * 8 * P], BF16, tag="wqk")
        wv_sb = persist.tile([P, 8 * 512], BF16, tag="wv")
        wpj = persist.tile([P, 4 * C], F32R, tag="wpj")
        bqk = persist.tile([P, 8], F32, tag="bqk")
        bias_v = persist.tile([P, 512], F32, tag="bias_v")

        xTs = [None] * NQC

        def load_x(ch):
            xT = xtp.tile([P, 8 * 512], BF16, tag="xT")
            for cb in range(8):
                nc.sync.dma_start(
                    xT[:, cb * 512 : (cb + 1) * 512],
                    xt_d[cb * P : (cb + 1) * P, ch * 512 : (ch + 1) * 512],
                )
            xTs[ch] = xT

        # ---- prologue: DMAs first so the PE can start ASAP ----
        load_x(0)
        nc.sync.dma_start(bvr[:], bv_d[:])
        nc.sync.dma_start(bqk[:], bqk_d[:].rearrange("(a p) -> p a", p=P))
        for chb in range(8):
            nc.sync.dma_start(
                wqk_sb[:, chb * 8 * P : (chb + 1) * 8 * P].rearrange(
                    "p (cb j) -> p cb j", cb=8
                ),
                wqk_d[:, chb * P : (chb + 1) * P].rearrange("(cb p) j -> p cb j", p=P),
            )
        nc.sync.dma_start(
            wv_sb[:].rearrange("p (cb j) -> p cb j", cb=8),
            wv_d[:].rearrange("(cb p) j -> p cb j", p=P),
        )
        nc.sync.dma_start(
            wpj[:].rearrange("r (pr j) -> r pr j", pr=4),
            wproj_d[:].rearrange("(pr r) j -> r pr j", r=P),
        )

        # big zero-fills on the otherwise-idle Pool engine
        make_upper_triangular(nc, mask[:], val=1.0, diag=True)
        for p_pair in range(4):
            nc.gpsimd.memset(qk_all[64:P, (3 * p_pair) * T : (3 * p_pair + 1) * T], 0.0)
            nc.gpsimd.memset(qk_all[0:64, (3 * p_pair + 1) * T : (3 * p_pair + 2) * T], 0.0)

        nc.vector.memset(ones_f[:], 1.0)
        nc.vector.tensor_copy(ones_t[:], ones_f[0:1, :])
        nc.vector.memset(selA[:].bitcast(F32), 0.0)
        nc.vector.memset(selB[:].bitcast(F32), 0.0)
        va4 = v_all[:].rearrange("p (a b c) -> p a b c", a=4, b=NKB, c=130)
        nc.vector.tensor_copy(va4[:, :, :, 64:65], ones_f[:, 0 : 4 * NKB])
        nc.vector.tensor_copy(va4[:, :, :, 129:130], ones_f[:, 0 : 4 * NKB])
        # selA row 32p: cols 0:64 = 1; selB row 32p: cols 64:128 = 1; else 0
        for pr in range(4):
            nc.sync.dma_start(
                selA[pr * 32 : pr * 32 + 1, 0:64].bitcast(F32), ones_f[0:1, 0:64]
            )
            nc.sync.dma_start(
                selB[pr * 32 : pr * 32 + 1, 64:P].bitcast(F32), ones_f[0:1, 0:64]
            )

        # ---- QKV building blocks ----
        def qkv_group_qk(ch, chb):
            xT = xTs[ch]
            p_pair, kind = chb // 2, chb % 2  # 0 = q block, 1 = k block
            pq = ps_x.tile([P, 512], F32, tag="px")
            for cb in range(8):
                nc.tensor.matmul(
                    pq[:],
                    wqk_sb[:, chb * 8 * P + cb * P : chb * 8 * P + (cb + 1) * P],
                    xT[:, cb * 512 : (cb + 1) * 512],
                    start=(cb == 0),
                    stop=(cb == 7),
                    skip_group_check=True,
                )
            t0 = ch * 512
            if kind == 0:  # q -> two zero-padded tiles
                blk_a, blk_b = 3 * p_pair, 3 * p_pair + 1
                nc.vector.tensor_scalar_add(
                    qk_all[0:64, blk_a * T + t0 : blk_a * T + t0 + 512],
                    pq[0:64, :],
                    bqk[0:64, chb : chb + 1],
                )
                nc.vector.tensor_scalar_add(
                    qk_all[64:P, blk_b * T + t0 : blk_b * T + t0 + 512],
                    pq[64:P, :],
                    bqk[64:P, chb : chb + 1],
                )
            else:  # k pair block
                blk = 3 * p_pair + 2
                nc.vector.tensor_scalar_add(
                    qk_all[:, blk * T + t0 : blk * T + t0 + 512],
                    pq[:],
                    bqk[:, chb : chb + 1],
                )

        def qkv_group_v(ch, tb):
            xT = xTs[ch]
            kb = ch * 4 + tb
            pv = ps_x.tile([P, 512], F32, tag="px")
            for cb in range(8):
                nc.tensor.matmul(
                    pv[:],
                    xT[:, cb * 512 + tb * P : cb * 512 + (tb + 1) * P],
                    wv_sb[:, cb * 512 : (cb + 1) * 512],
                    start=(cb == 0),
                    stop=(cb == 7),
                    skip_group_check=True,
                )
            dst = bass.AP(
                v_all[:].tensor,
                v_all[:].offset + kb * 130,
                [[v_all[:].ap[0][0], P], [NKB * 130, 4], [65, 2], [1, 64]],
            )
            src = bass.AP(
                pv[:].tensor,
                pv[:].offset,
                [[pv[:].ap[0][0], P], [128, 4], [64, 2], [1, 64]],
            )
            bsrc = bass.AP(
                bias_v[:].tensor,
                bias_v[:].offset,
                [[bias_v[:].ap[0][0], P], [128, 4], [64, 2], [1, 64]],
            )
            nc.vector.tensor_tensor(dst, src, bsrc, mybir.AluOpType.add)

        # ---- normalization (deferred one round) ----
        zras = [None] * NQC
        zrbs = [None] * NQC

        def norm_unit(qc, prs):
            rz2a = rzap.tile([P, 512], F32R, tag="rz2a")
            rz2b = rzap.tile([P, 512], F32R, tag="rz2b")
            with nc.allow_low_precision(reason="1/Z broadcast feeds f32r matmul"):
                for pr in prs:
                    nc.vector.reciprocal(
                        rz2a[pr * 32 : pr * 32 + 1, :],
                        zras[qc][pr * 32 : pr * 32 + 1, :],
                    )
                    nc.vector.reciprocal(
                        rz2b[pr * 32 : pr * 32 + 1, :],
                        zrbs[qc][pr * 32 : pr * 32 + 1, :],
                    )
            for pr in prs:
                col = pr * T + qc * 512
                pbt = ps_x.tile([P, 512], F32, tag="px")
                nc.tensor.matmul(
                    pbt[:],
                    selA[pr * 32 : pr * 32 + 1, :],
                    rz2a[pr * 32 : pr * 32 + 1, :],
                    start=True, stop=False,
                    tile_position=(pr * 32, 0),
                    skip_group_check=True,
                )
                nc.tensor.matmul(
                    pbt[:],
                    selB[pr * 32 : pr * 32 + 1, :],
                    rz2b[pr * 32 : pr * 32 + 1, :],
                    start=False, stop=True,
                    tile_position=(pr * 32, 0),
                    skip_group_check=True,
                )
                nc.vector.tensor_mul(
                    aoT[:, col : col + 512], aoT[:, col : col + 512], pbt[:]
                )

        # ---- projection ----
        proj_ctr = [0]

        def proj_group(qc, tb, oc):
            py = ps_x.tile([P, 512], F32, tag="px")
            for pp in range(4):
                nc.tensor.matmul(
                    py[:],
                    aoT[:, pp * T + tb * P : pp * T + (tb + 1) * P],
                    wpj[:, pp * C + oc * 512 : pp * C + (oc + 1) * 512],
                    start=(pp == 0),
                    stop=(pp == 3),
                    skip_group_check=True,
                )
            ys = ysp.tile([P, 512], F32, tag="ys")
            nc.vector.tensor_copy(ys[:], py[:])
            proj_ctr[0] += 1
            nc.sync.dma_start(
                y_d[tb * P : (tb + 1) * P, oc * 512 : (oc + 1) * 512], ys[:]
            )

        # ---- attention units ----
        def attn_scores(qc, p_pair, kb):
            """Scores for both heads into one [128,1024] PSUM tile + one/two exps."""
            kblk = 3 * p_pair + 2
            qoff = max(0, kb * P - qc * 512)
            diag = kb * P >= qc * 512
            ps = ps_s.tile([P, 1024], F32, tag="ps")
            at = atp.tile([P, 1024], BF16, tag="at")
            for hh in range(2):
                qblk = 3 * p_pair + hh
                nc.tensor.matmul(
                    ps[:, hh * 512 + qoff : (hh + 1) * 512],
                    qk_all[:, kblk * T + kb * P : kblk * T + (kb + 1) * P],
                    qk_all[:, qblk * T + qc * 512 + qoff : qblk * T + (qc + 1) * 512],
                    start=True,
                    stop=True,
                    skip_group_check=True,
                )
            if qoff == 0:
                nc.scalar.activation(at[:, 0:1024], ps[:, 0:1024], Exp, scale=SCALE)
            else:
                for hh in range(2):
                    nc.scalar.activation(
                        at[:, hh * 512 + qoff : (hh + 1) * 512],
                        ps[:, hh * 512 + qoff : (hh + 1) * 512],
                        Exp,
                        scale=SCALE,
                    )
            if diag:
                for hh in range(2):
                    nc.vector.tensor_mul(
                        at[:, hh * 512 + qoff : hh * 512 + qoff + P],
                        at[:, hh * 512 + qoff : hh * 512 + qoff + P],
                        mask[:],
                    )
            return (qc, p_pair, kb, at, qoff)

        def attn_v(unit, po, nkb):
            qc, p_pair, kb, at, qoff = unit
            for hh in range(2):
                nc.tensor.matmul(
                    po[hh][:, qoff:512],
                    v_all[:, p_pair * NKB * 130 + kb * 130 + hh * 65 :
                          p_pair * NKB * 130 + kb * 130 + hh * 65 + 65],
                    at[:, hh * 512 + qoff : (hh + 1) * 512],
                    start=(kb == 0),
                    stop=(kb == nkb - 1),
                    skip_group_check=True,
                )
            if kb == nkb - 1:
                # pair-end: evict raw ao rows + Z rows to SBUF staging
                col = p_pair * T + qc * 512
                nc.vector.tensor_copy(aoT[0:64, col : col + 512], po[0][0:64, :])
                nc.vector.tensor_copy(aoT[64:P, col : col + 512], po[1][0:64, :])
                nc.vector.tensor_copy(
                    zras[qc][p_pair * 32 : p_pair * 32 + 1, :], po[0][64:65, :]
                )
                nc.vector.tensor_copy(
                    zrbs[qc][p_pair * 32 : p_pair * 32 + 1, :], po[1][64:65, :]
                )

        # ---- fused main loop ----
        # Prologue: QKV for chunk 0 (no attention to hide behind).
        for chb in range(8):
            qkv_group_qk(0, chb)
        # bias_v[128, 512] = b_v broadcast along partitions (K=1 matmul)
        pbv = ps_x.tile([P, 512], F32, tag="px", name="pbv")
        nc.tensor.matmul(pbv[:], ones_t[:], bvr[:], start=True, stop=True,
                         skip_group_check=True)
        nc.vector.tensor_copy(bias_v[:], pbv[:])
        for tb in range(4):
            qkv_group_v(0, tb)

        pipe = deque()   # pending attn_v emissions
        DEPTH = 3

        for qc in range(NQC):
            zras[qc] = zsp_p.tile([P, 512], F32, tag="zra", name=f"zra{qc}")
            zrbs[qc] = zsp_p.tile([P, 512], F32, tag="zrb", name=f"zrb{qc}")
            if qc < NQC - 1:
                load_x(qc + 1)
            fill = deque()
            if qc >= 1:
                fill.append(lambda q=qc - 1: norm_unit(q, (0, 1, 2, 3)))
            if qc < NQC - 1:
                for chb in range(8):
                    fill.append(lambda c=qc + 1, b=chb: qkv_group_qk(c, b))
                for tb in range(4):
                    fill.append(lambda c=qc + 1, b=tb: qkv_group_v(c, b))
            if qc >= 1:
                for tb in range((qc - 1) * 4, qc * 4):
                    for oc in range(2):
                        fill.append(lambda t=tb, o=oc: proj_group(qc - 1, t, o))

            nkb = 4 * qc + 4
            units = 4 * nkb
            popped = 0
            ui = 0
            for p_pair in range(4):
                po = [
                    ps_o.tile([65, 512], F32, tag="po0", name="po0"),
                    ps_o.tile([65, 512], F32, tag="po1", name="po1"),
                ]
                for kb in range(nkb):
                    pipe.append((attn_scores(qc, p_pair, kb), po, nkb))
                    # pace fillers evenly across the round's units
                    ui += 1
                    want = (len(fill) + popped) * ui // units
                    while popped < want and fill:
                        fill.popleft()()
                        popped += 1
                    if len(pipe) > DEPTH:
                        u, upo, unkb = pipe.popleft()
                        attn_v(u, upo, unkb)
                # final q chunk: normalize each pair as soon as its Z is out
                if qc == NQC - 1 and p_pair >= 1:
                    while pipe:
                        u, upo, unkb = pipe.popleft()
                        attn_v(u, upo, unkb)
                    norm_unit(qc, (0, 1) if p_pair == 1 else (p_pair,))
            while fill:
                fill.popleft()()
            while pipe:
                u, upo, unkb = pipe.popleft()
                attn_v(u, upo, unkb)

        # final tail: project the last chunk
        for tb in range((NQC - 1) * 4, NQC * 4):
            for oc in range(2):
                proj_group(NQC - 1, tb, oc)

    nc.compile()
    return nc


def _shard_inputs(x, W_qkv, b_qkv, W_proj):
    """Build the 8 per-core input maps."""
    in_maps = []
    for c in range(8):
        b = c // 2
        hg = c % 2
        heads = [hg * 8 + j for j in range(8)]
        qk_cols = []
        for p in range(4):
            ha, hb = heads[2 * p], heads[2 * p + 1]
            for part in range(2):  # q, k
                qk_cols.extend(range(ha * 192 + part * 64, ha * 192 + part * 64 + 64))
                qk_cols.extend(range(hb * 192 + part * 64, hb * 192 + part * 64 + 64))
        qk_cols = np.array(qk_cols)
        v_cols = []
        for p in range(4):
            ha, hb = heads[2 * p], heads[2 * p + 1]
            v_cols.extend(range(ha * 192 + 128, ha * 192 + 192))
            v_cols.extend(range(hb * 192 + 128, hb * 192 + 192))
        v_cols = np.array(v_cols)
        in_maps.append(
            {
                "xt": np.ascontiguousarray(x[b].T.astype(ml_dtypes.bfloat16)),
                "wqk": np.ascontiguousarray(W_qkv[:, qk_cols].astype(ml_dtypes.bfloat16)),
                "bqk": np.ascontiguousarray(b_qkv[qk_cols], dtype=np.float32),
                "wv": np.ascontiguousarray(W_qkv[:, v_cols].astype(ml_dtypes.bfloat16)),
                "bv": np.ascontiguousarray(
                    b_qkv[v_cols].reshape(1, 512), dtype=np.float32
                ),
                "wproj": np.ascontiguousarray(
                    W_proj[hg * 512 : (hg + 1) * 512, :], dtype=np.float32
                ),
            }
        )
    return in_maps


_NC = None


def kernel(x, W_qkv, b_qkv, W_proj, b_proj, _trace=False):
    global _NC
    x = np.asarray(x, dtype=np.float32)
    W_qkv = np.asarray(W_qkv, dtype=np.float32)
    b_qkv = np.asarray(b_qkv, dtype=np.float32)
    W_proj = np.asarray(W_proj, dtype=np.float32)
    b_proj = np.asarray(b_proj, dtype=np.float32)

    in_maps = _shard_inputs(x, W_qkv, b_qkv, W_proj)
    if _NC is None:
        _NC = build_kernel()
    res = run_bass_kernel_spmd(
        _NC, in_maps, core_ids=list(range(8)), trace=_trace,
        trace_cores=list(range(8)) if _trace else None,
    )
    out = np.empty((B, T, C), dtype=np.float32)
    for b in range(B):
        out[b] = res.results[2 * b]["y"] + res.results[2 * b + 1]["y"] + b_proj
    if _trace:
        return out, res
    return out


# revision 26
# speedup vs baseline: 1.1525x; 1.1525x over previous
"""Multi-head causal self-attention (B=4, T=2048, C=1024, H=16) on 8 TRN2 cores.

Sharding: core c handles batch b = c//2 and head-group hg = c%2 (8 heads):
data parallel over B, tensor parallel over H. Each core computes qk^T for its
heads (xT @ Wqk column-slice, transposed per-head-pair layout), V in natural
layout, causal attention for its 8 heads, and a partial output projection
(row-split W_proj) -> y_partial [T, C]. Host transposes x per core and sums
y[b] = y_partial[2b] + y_partial[2b+1] + b_proj.

Single fused pipeline: the QKV projection for token chunk r+1 is interleaved
as filler work into attention round r (one round per 512-token q chunk), so
the tensor engine never drains between "phases" and stays at max p-state.
Scores for both heads of a pair go into one [128,1024] PSUM tile and share
one exp activation; attn@V emission is software-pipelined two units behind
the score matmuls so it never waits on the scalar engine. Z rows are DMA-
spread straight out of PSUM; normalization uses a 128-lane reciprocal and a
K=2 broadcast matmul, deferred one round and overlapped with the next round.
"""

from collections import deque
from contextlib import ExitStack

import ml_dtypes
import numpy as np

import concourse.bass as bass
import concourse.bacc as bacc
import concourse.mybir as mybir
import concourse.tile as tile
from concourse.bass_utils import run_bass_kernel_spmd
from concourse.masks import make_upper_triangular

B, T, C, H, HS = 4, 2048, 1024, 16, 64
P = 128
NQC = T // 512          # q-chunks of 512
NKB = T // P            # key blocks of 128
SCALE = HS ** -0.5

F32 = mybir.dt.float32
F32R = mybir.dt.float32r
BF16 = mybir.dt.bfloat16
Exp = mybir.ActivationFunctionType.Exp


def build_kernel():
    nc = bacc.Bacc("TRN2", target_bir_lowering=False)

    xt_d = nc.dram_tensor("xt", (C, T), BF16, kind="ExternalInput")
    wqk_d = nc.dram_tensor("wqk", (C, 8 * P), BF16, kind="ExternalInput")
    bqk_d = nc.dram_tensor("bqk", (8 * P,), F32, kind="ExternalInput")
    wv_d = nc.dram_tensor("wv", (C, 512), BF16, kind="ExternalInput")
    bv_d = nc.dram_tensor("bv", (1, 512), F32R, kind="ExternalInput")
    wproj_d = nc.dram_tensor("wproj", (8 * HS, C), F32R, kind="ExternalInput")
    y_d = nc.dram_tensor("y", (T, C), F32, kind="ExternalOutput")

    with tile.TileContext(nc) as tc, ExitStack() as big:
        const = big.enter_context(tc.tile_pool(name="const", bufs=1))
        persist = big.enter_context(tc.tile_pool(name="persist", bufs=1))
        xtp = big.enter_context(tc.tile_pool(name="xtp", bufs=2))
        atp = big.enter_context(tc.tile_pool(name="atp", bufs=5))
        zsp_p = big.enter_context(tc.tile_pool(name="zsp_p", bufs=2))
        rzap = big.enter_context(tc.tile_pool(name="rzap", bufs=2))
        ysp = big.enter_context(tc.tile_pool(name="ysp", bufs=3))
        ps_s = big.enter_context(tc.tile_pool(name="ps_s", bufs=2, space="PSUM"))
        ps_o = big.enter_context(tc.tile_pool(name="ps_o", bufs=1, space="PSUM"))
        ps_x = big.enter_context(tc.tile_pool(name="ps_x", bufs=2, space="PSUM"))

        # ---- tiles ----
        mask = const.tile([P, P], BF16, tag="mask")
        ones_f = const.tile([P, P], F32, tag="ones_f")
        ones_t = const.tile([1, P], F32R, tag="ones")
        sel2 = const.tile([P, P], F32R, tag="sel2")
        bvr = const.tile([1, 512], F32R, tag="bvr")
        # qk_all: 12 blocks of [128, T] bf16; per pair p:
        #   block 3p   = qpadA: rows 0:64 q of head 2p, rows 64:128 zero
        #   block 3p+1 = qpadB: rows 0:64 zero, rows 64:128 q of head 2p+1
        #   block 3p+2 = k pair: rows 0:64 k(2p), 64:128 k(2p+1)
        qk_all = persist.tile([P, 12 * T], BF16, tag="qk")
        # v_all: per (pair, kb): [vA(64) | onesA(1) | vB(64) | onesB(1)] = 130
        v_all = persist.tile([P, 4 * NKB * 130], BF16, tag="v")
        # aoT: pair-stacked [128 = ch(head 2p) | ch(head 2p+1), 4 * T]
        aoT = persist.tile([P, 4 * T], F32R, tag="aoT")
        wqk_sb = persist.tile([P, 8 * 8 * P], BF16, tag="wqk")
        wv_sb = persist.tile([P, 8 * 512], BF16, tag="wv")
        wpj = persist.tile([P, 4 * C], F32R, tag="wpj")
        bqk = persist.tile([P, 8], F32, tag="bqk")
        bias_v = persist.tile([P, 512], F32, tag="bias_v")

        xTs = [None] * NQC

        def load_x(ch):
            xT = xtp.tile([P, 8 * 512], BF16, tag="xT")
            for cb in range(8):
                nc.sync.dma_start(
                    xT[:, cb * 512 : (cb + 1) * 512],
                    xt_d[cb * P : (cb + 1) * P, ch * 512 : (ch + 1) * 512],
                )
            xTs[ch] = xT

        # ---- prologue: DMAs first so the PE can start ASAP ----
        load_x(0)
        nc.sync.dma_start(bvr[:], bv_d[:])
        nc.sync.dma_start(bqk[:], bqk_d[:].rearrange("(a p) -> p a", p=P))
        for chb in range(8):
            nc.sync.dma_start(
                wqk_sb[:, chb * 8 * P : (chb + 1) * 8 * P].rearrange(
                    "p (cb j) -> p cb j", cb=8
                ),
                wqk_d[:, chb * P : (chb + 1) * P].rearrange("(cb p) j -> p cb j", p=P),
            )
        nc.sync.dma_start(
            wv_sb[:].rearrange("p (cb j) -> p cb j", cb=8),
            wv_d[:].rearrange("(cb p) j -> p cb j", p=P),
        )
        nc.sync.dma_start(
            wpj[:].rearrange("r (pr j) -> r pr j", pr=4),
            wproj_d[:].rearrange("(pr r) j -> r pr j", r=P),
        )

        # big zero-fills on the otherwise-idle Pool engine
        make_upper_triangular(nc, mask[:], val=1.0, diag=True)
        for p_pair in range(4):
            nc.gpsimd.memset(qk_all[64:P, (3 * p_pair) * T : (3 * p_pair + 1) * T], 0.0)
            nc.gpsimd.memset(qk_all[0:64, (3 * p_pair + 1) * T : (3 * p_pair + 2) * T], 0.0)

        nc.vector.memset(ones_f[:], 1.0)
        nc.vector.tensor_copy(ones_t[:], ones_f[0:1, :])
        nc.vector.memset(sel2[:].bitcast(F32), 0.0)
        va4 = v_all[:].rearrange("p (a b c) -> p a b c", a=4, b=NKB, c=130)
        nc.vector.tensor_copy(va4[:, :, :, 64:65], ones_f[:, 0 : 4 * NKB])
        nc.vector.tensor_copy(va4[:, :, :, 129:130], ones_f[:, 0 : 4 * NKB])
        # sel2: rows {32p: cols 0:64 = 1}, {32p+1: cols 64:128 = 1}, else 0
        for pr in range(4):
            nc.sync.dma_start(
                sel2[pr * 32 : pr * 32 + 1, 0:64].bitcast(F32), ones_f[0:1, 0:64]
            )
            nc.sync.dma_start(
                sel2[pr * 32 + 1 : pr * 32 + 2, 64:P].bitcast(F32), ones_f[0:1, 0:64]
            )

        # ---- QKV building blocks ----
        def qkv_group_qk(ch, chb):
            xT = xTs[ch]
            p_pair, kind = chb // 2, chb % 2  # 0 = q block, 1 = k block
            pq = ps_x.tile([P, 512], F32, tag="px")
            for cb in range(8):
                nc.tensor.matmul(
                    pq[:],
                    wqk_sb[:, chb * 8 * P + cb * P : chb * 8 * P + (cb + 1) * P],
                    xT[:, cb * 512 : (cb + 1) * 512],
                    start=(cb == 0),
                    stop=(cb == 7),
                    skip_group_check=True,
                )
            t0 = ch * 512
            if kind == 0:  # q -> two zero-padded tiles
                blk_a, blk_b = 3 * p_pair, 3 * p_pair + 1
                nc.vector.tensor_scalar_add(
                    qk_all[0:64, blk_a * T + t0 : blk_a * T + t0 + 512],
                    pq[0:64, :],
                    bqk[0:64, chb : chb + 1],
                )
                nc.vector.tensor_scalar_add(
                    qk_all[64:P, blk_b * T + t0 : blk_b * T + t0 + 512],
                    pq[64:P, :],
                    bqk[64:P, chb : chb + 1],
                )
            else:  # k pair block
                blk = 3 * p_pair + 2
                nc.vector.tensor_scalar_add(
                    qk_all[:, blk * T + t0 : blk * T + t0 + 512],
                    pq[:],
                    bqk[:, chb : chb + 1],
                )

        def qkv_group_v(ch, tb):
            xT = xTs[ch]
            kb = ch * 4 + tb
            pv = ps_x.tile([P, 512], F32, tag="px")
            for cb in range(8):
                nc.tensor.matmul(
                    pv[:],
                    xT[:, cb * 512 + tb * P : cb * 512 + (tb + 1) * P],
                    wv_sb[:, cb * 512 : (cb + 1) * 512],
                    start=(cb == 0),
                    stop=(cb == 7),
                    skip_group_check=True,
                )
            dst = bass.AP(
                v_all[:].tensor,
                v_all[:].offset + kb * 130,
                [[v_all[:].ap[0][0], P], [NKB * 130, 4], [65, 2], [1, 64]],
            )
            src = bass.AP(
                pv[:].tensor,
                pv[:].offset,
                [[pv[:].ap[0][0], P], [128, 4], [64, 2], [1, 64]],
            )
            bsrc = bass.AP(
                bias_v[:].tensor,
                bias_v[:].offset,
                [[bias_v[:].ap[0][0], P], [128, 4], [64, 2], [1, 64]],
            )
            nc.vector.tensor_tensor(dst, src, bsrc, mybir.AluOpType.add)

        # ---- normalization (deferred one round) ----
        zras = [None] * NQC
        zrbs = [None] * NQC

        def norm_unit(qc, prs):
            # spread Z rows across 128 partitions, reciprocal, unspread
            lo = min(prs) * 32
            hi = (max(prs) + 1) * 32
            zsp = zsp_p.tile([P, 32], F32, tag="zsp")
            for pr in prs:
                for hh in range(2):
                    r = pr * 2 + hh
                    srcz = (zras if hh == 0 else zrbs)[qc][pr * 32 : pr * 32 + 1, :]
                    nc.sync.dma_start(zsp[r * 16 : (r + 1) * 16, :], srcz)
            zspr = zsp_p.tile([P, 32], F32, tag="zspr")
            nc.vector.reciprocal(zspr[lo:hi, :], zsp[lo:hi, :])
            rz2 = rzap.tile([P, 512], F32R, tag="rz2")
            for pr in prs:
                for hh in range(2):
                    r = pr * 2 + hh
                    nc.sync.dma_start(
                        rz2[pr * 32 + hh : pr * 32 + hh + 1, :].bitcast(F32),
                        zspr[r * 16 : (r + 1) * 16, :],
                    )
            for pr in prs:
                col = pr * T + qc * 512
                pbt = ps_x.tile([P, 512], F32, tag="px")
                nc.tensor.matmul(
                    pbt[:],
                    sel2[pr * 32 : pr * 32 + 2, :],
                    rz2[pr * 32 : pr * 32 + 2, :],
                    start=True, stop=True,
                    tile_position=(pr * 32, 0),
                    skip_group_check=True,
                )
                nc.vector.tensor_mul(
                    aoT[:, col : col + 512], aoT[:, col : col + 512], pbt[:]
                )

        # ---- projection ----
        proj_ctr = [0]

        def proj_group(qc, tb, oc):
            py = ps_x.tile([P, 512], F32, tag="px")
            for pp in range(4):
                nc.tensor.matmul(
                    py[:],
                    aoT[:, pp * T + tb * P : pp * T + (tb + 1) * P],
                    wpj[:, pp * C + oc * 512 : pp * C + (oc + 1) * 512],
                    start=(pp == 0),
                    stop=(pp == 3),
                    skip_group_check=True,
                )
            ys = ysp.tile([P, 512], F32, tag="ys")
            nc.vector.tensor_copy(ys[:], py[:])
            proj_ctr[0] += 1
            nc.sync.dma_start(
                y_d[tb * P : (tb + 1) * P, oc * 512 : (oc + 1) * 512], ys[:]
            )

        # ---- attention units ----
        def attn_scores(qc, p_pair, kb):
            """Scores for both heads into one [128,1024] PSUM tile + one/two exps."""
            kblk = 3 * p_pair + 2
            qoff = max(0, kb * P - qc * 512)
            diag = kb * P >= qc * 512
            ps = ps_s.tile([P, 1024], F32, tag="ps")
            at = atp.tile([P, 1024], BF16, tag="at")
            for hh in range(2):
                qblk = 3 * p_pair + hh
                nc.tensor.matmul(
                    ps[:, hh * 512 + qoff : (hh + 1) * 512],
                    qk_all[:, kblk * T + kb * P : kblk * T + (kb + 1) * P],
                    qk_all[:, qblk * T + qc * 512 + qoff : qblk * T + (qc + 1) * 512],
                    start=True,
                    stop=True,
                    skip_group_check=True,
                )
            if qoff == 0:
                nc.scalar.activation(at[:, 0:1024], ps[:, 0:1024], Exp, scale=SCALE)
            else:
                for hh in range(2):
                    nc.scalar.activation(
                        at[:, hh * 512 + qoff : (hh + 1) * 512],
                        ps[:, hh * 512 + qoff : (hh + 1) * 512],
                        Exp,
                        scale=SCALE,
                    )
            if diag:
                for hh in range(2):
                    nc.vector.tensor_mul(
                        at[:, hh * 512 + qoff : hh * 512 + qoff + P],
                        at[:, hh * 512 + qoff : hh * 512 + qoff + P],
                        mask[:],
                    )
            return (qc, p_pair, kb, at, qoff)

        def attn_v(unit, po, nkb):
            qc, p_pair, kb, at, qoff = unit
            for hh in range(2):
                nc.tensor.matmul(
                    po[hh][:, qoff:512],
                    v_all[:, p_pair * NKB * 130 + kb * 130 + hh * 65 :
                          p_pair * NKB * 130 + kb * 130 + hh * 65 + 65],
                    at[:, hh * 512 + qoff : (hh + 1) * 512],
                    start=(kb == 0),
                    stop=(kb == nkb - 1),
                    skip_group_check=True,
                )
            if kb == nkb - 1:
                # pair-end: evict raw ao rows + Z rows to SBUF staging
                col = p_pair * T + qc * 512
                nc.vector.tensor_copy(aoT[0:64, col : col + 512], po[0][0:64, :])
                nc.scalar.copy(aoT[64:P, col : col + 512], po[1][0:64, :])
                nc.vector.tensor_copy(
                    zras[qc][p_pair * 32 : p_pair * 32 + 1, :], po[0][64:65, :]
                )
                nc.vector.tensor_copy(
                    zrbs[qc][p_pair * 32 : p_pair * 32 + 1, :], po[1][64:65, :]
                )

        # ---- fused main loop ----
        # Prologue: QKV for chunk 0 (no attention to hide behind).
        for chb in range(8):
            qkv_group_qk(0, chb)
        # bias_v[128, 512] = b_v broadcast along partitions (K=1 matmul)
        pbv = ps_x.tile([P, 512], F32, tag="px", name="pbv")
        nc.tensor.matmul(pbv[:], ones_t[:], bvr[:], start=True, stop=True,
                         skip_group_check=True)
        nc.vector.tensor_copy(bias_v[:], pbv[:])
        for tb in range(4):
            qkv_group_v(0, tb)

        pipe = deque()   # pending attn_v emissions
        DEPTH = 3

        for qc in range(NQC):
            zras[qc] = zsp_p.tile([P, 512], F32, tag="zra", name=f"zra{qc}")
            zrbs[qc] = zsp_p.tile([P, 512], F32, tag="zrb", name=f"zrb{qc}")
            if qc < NQC - 1:
                load_x(qc + 1)
            fill = deque()
            if qc >= 1:
                fill.append(lambda q=qc - 1: norm_unit(q, (0, 1, 2, 3)))
            if qc < NQC - 1:
                for chb in range(8):
                    fill.append(lambda c=qc + 1, b=chb: qkv_group_qk(c, b))
                for tb in range(4):
                    fill.append(lambda c=qc + 1, b=tb: qkv_group_v(c, b))
            if qc >= 1:
                for tb in range((qc - 1) * 4, qc * 4):
                    for oc in range(2):
                        fill.append(lambda t=tb, o=oc: proj_group(qc - 1, t, o))

            nkb = 4 * qc + 4
            units = 4 * nkb
            popped = 0
            ui = 0
            for p_pair in range(4):
                po = [
                    ps_o.tile([65, 512], F32, tag="po0", name="po0"),
                    ps_o.tile([65, 512], F32, tag="po1", name="po1"),
                ]
                for kb in range(nkb):
                    pipe.append((attn_scores(qc, p_pair, kb), po, nkb))
                    # pace fillers evenly across the round's units
                    ui += 1
                    want = (len(fill) + popped) * ui // units
                    while popped < want and fill:
                        fill.popleft()()
                        popped += 1
                    if len(pipe) > DEPTH:
                        u, upo, unkb = pipe.popleft()
                        attn_v(u, upo, unkb)
                # final q chunk: normalize each pair as soon as its Z is out
                if qc == NQC - 1 and p_pair >= 1:
                    while pipe:
                        u, upo, unkb = pipe.popleft()
                        attn_v(u, upo, unkb)
                    norm_unit(qc, (0, 1) if p_pair == 1 else (p_pair,))
            while fill:
                fill.popleft()()
            while pipe:
                u, upo, unkb = pipe.popleft()
                attn_v(u, upo, unkb)

        # final tail: project the last chunk
        for tb in range((NQC - 1) * 4, NQC * 4):
            for oc in range(2):
                proj_group(NQC - 1, tb, oc)

    nc.compile()
    return nc


def _shard_inputs(x, W_qkv, b_qkv, W_proj):
    """Build the 8 per-core input maps."""
    in_maps = []
    for c in range(8):
        b = c // 2
        hg = c % 2
        heads = [hg * 8 + j for j in range(8)]
        qk_cols = []
        for p in range(4):
            ha, hb = heads[2 * p], heads[2 * p + 1]
            for part in range(2):  # q, k
                qk_cols.extend(range(ha * 192 + part * 64, ha * 192 + part * 64 + 64))
                qk_cols.extend(range(hb * 192 + part * 64, hb * 192 + part * 64 + 64))
        qk_cols = np.array(qk_cols)
        v_cols = []
        for p in range(4):
            ha, hb = heads[2 * p], heads[2 * p + 1]
            v_cols.extend(range(ha * 192 + 128, ha * 192 + 192))
            v_cols.extend(range(hb * 192 + 128, hb * 192 + 192))
        v_cols = np.array(v_cols)
        in_maps.append(
            {
                "xt": np.ascontiguousarray(x[b].T.astype(ml_dtypes.bfloat16)),
                "wqk": np.ascontiguousarray(W_qkv[:, qk_cols].astype(ml_dtypes.bfloat16)),
                "bqk": np.ascontiguousarray(b_qkv[qk_cols], dtype=np.float32),
                "wv": np.ascontiguousarray(W_qkv[:, v_cols].astype(ml_dtypes.bfloat16)),
                "bv": np.ascontiguousarray(
                    b_qkv[v_cols].reshape(1, 512), dtype=np.float32
                ),
                "wproj": np.ascontiguousarray(
                    W_proj[hg * 512 : (hg + 1) * 512, :], dtype=np.float32
                ),
            }
        )
    return in_maps


_NC = None


def kernel(x, W_qkv, b_qkv, W_proj, b_proj, _trace=False):
    global _NC
    x = np.asarray(x, dtype=np.float32)
    W_qkv = np.asarray(W_qkv, dtype=np.float32)
    b_qkv = np.asarray(b_qkv, dtype=np.float32)
    W_proj = np.asarray(W_proj, dtype=np.float32)
    b_proj = np.asarray(b_proj, dtype=np.float32)

    in_maps = _shard_inputs(x, W_qkv, b_qkv, W_proj)
    if _NC is None:
        _NC = build_kernel()
    res = run_bass_kernel_spmd(
        _NC, in_maps, core_ids=list(range(8)), trace=_trace,
        trace_cores=list(range(8)) if _trace else None,
    )
    out = np.empty((B, T, C), dtype=np.float32)
    for b in range(B):
        out[b] = res.results[2 * b]["y"] + res.results[2 * b + 1]["y"] + b_proj
    if _trace:
        return out, res
    return out
